# revision 75
# baseline (speedup 1.0000x reference)
"""Block-sparse (DeepSpeed fixed-layout) causal self-attention on 8 trn2 NeuronCores.

Problem: B=2, H=16, L=2048, D=64, fp32; BLOCK=16, STRIDE=64, NUMVERTS=1, VERTSIZE=1.
Layout per head (identical for all heads since numverts=1):
  - intra-window block-causal attention within each 64-token window (4 blocks of 16)
  - "summary" attention: every query attends the last 16 tokens (block col 3) of
    every earlier 64-token window (earlier *pair* via summary path; the immediately
    preceding window within the same 128-pair is covered by the local path).

Strategy (per core; 32 (b,h) pairs sharded 4 per core, no collectives):
  S^T dataflow: St[k,q] = lhsT.T @ rhs, keys on PSUM partitions.
  Summary QK: chunk s = host-gathered summary K^T of windows 8s..8s+7 (128 keys),
    contraction rows [0:64] (no mask rows needed: chunks are fully allowed vs all
    later query groups). The group==chunk diagonal gets a pair-causal mask added
    as one rank-8 matmul from tiny constant tiles (dsel one-hot x dval values).
  Local QK: window-pairs (128 keys x 128 queries), contraction [0:72] where rows
    64-71 carry the rank-8 local causal mask (selector rows on K cols, value
    rows on Q cols).
  exp() on ScalarE (the bottleneck engine), 7 instructions per (b,h),
    reading two ping-pong summary PSUM tiles (A/B) plus a local tile; the
    diag pieces' first 128 query cols (fully pair-causal-masked, exp == 0)
    are skipped in QK, exp, and AV alike. Piece-to-tile mapping is chosen so
    each tile's last reader retires early and the next bh's QK matmuls
    overlap the current bh's exps. ACT runs gap-free back-to-back in steady
    state (the span == ACT busy).
  AV transposed: out[c,q] = Vpx.T @ Et with Vpx = [V | 1] stationary [128k,65]
    and Et moving -- column 64 of the [65, 512] PSUM tile is the softmax
    denominator l[q]. Chunk-0 opens the bank full-width (start=True zeroes at
    2KB-bank granularity), everything else accumulates. avg2/avg3 of each bh
    are emitted after the NEXT bh's s0 pieces (software pipelining) so PE
    prioritizes feeding ACT.
  DVE copies [65,512] PSUM fp32 -> fp16 SBUF; per-group output DMAs on the
    last bh only (shorter drain tail), one merged DMA per earlier bh.
  Startup: ScalarE exp-table load + PE clock-ramp warmup matmuls run during
    the initial input DMAs. Host does the final O = O_unnorm / l divide +
    transpose (free), like all other layout work.
"""

import numpy as np

# ---------------- problem constants (hardcoded per contract) ----------------
B, H, L, D = 2, 16, 2048, 64
BLOCK = 16
WIN = 64              # stride window (tokens)
NWIN = L // WIN       # 32 windows
NSUM = NWIN * BLOCK   # 512 summary keys (last 16 tokens of each window)
NG = 4                # query groups per sequence
GQ = L // NG          # 512 queries per group
NCORES = 8
NBH = (B * H) // NCORES  # 4 (b,h) per core
MASKVAL = -30000.0

_SUMIDX = np.array([64 * m + 48 + j for m in range(NWIN) for j in range(BLOCK)])


def _host_masks():
    """Constant mask tiles, fp16.

    mq8 [8, L]  : local mask VALUE rows (Q side), window-pair periodic.
    mk8 [8, L]  : local mask SELECTOR rows (K side), one-hot key 16-block in pair.
    dsel [8,128]: diag-chunk selector, one-hot of key 16-block within chunk.
    dval [8,512]: diag-chunk values, MASKVAL iff query pair <= key-window pair.
    """
    qc = np.arange(L)
    jj = qc % 128
    ap = jj // WIN
    rp = (jj % WIN) // BLOCK
    mq8 = np.zeros((8, L), np.float32)
    for i in range(8):
        a, b = i // 4, i % 4
        active = ((a == ap) & (b <= rp)) | ((a == 0) & (ap == 1) & (b == 3))
        mq8[i] = np.where(active, 0.0, MASKVAL)
    kc = np.arange(L)
    mk8 = np.zeros((8, L), np.float32)
    for i in range(8):
        mk8[i] = ((kc % 128) // BLOCK == i).astype(np.float32)
    dsel = np.zeros((8, 128), np.float32)
    for r in range(8):
        dsel[r] = (np.arange(128) // BLOCK == r).astype(np.float32)
    dval = np.zeros((8, GQ), np.float32)
    qg = np.arange(GQ)
    for r in range(8):
        dval[r] = np.where((qg // 128) <= (r // 2), MASKVAL, 0.0)
    return (mq8.astype(np.float16), mk8.astype(np.float16),
            dsel.astype(np.float16), dval.astype(np.float16))


# ---------------- device program ----------------
_NC_CACHE = {}


def _build_nc(reps=1):
    if ("nc", reps) in _NC_CACHE:
        return _NC_CACHE[("nc", reps)]
    from contextlib import ExitStack

    import concourse.bacc as bacc
    import concourse.tile as tile
    from concourse import mybir

    F16 = mybir.dt.float16
    F32 = mybir.dt.float32
    EXP = mybir.ActivationFunctionType.Exp

    nc = bacc.Bacc("TRN2", target_bir_lowering=False)

    # qkt = [gathered summary K^T | Q^T/8 | K^T] along cols, 64 d-rows
    # (summary-K first so bh0's piece-1 DMA = exactly what s0b needs)
    qkt_d = nc.dram_tensor(
        "qkt", [NBH, 64, 2 * L + NSUM], F16, kind="ExternalInput"
    )
    # vpx = [V|1] reshaped (16 local 128-key tiles) ++ gathered summary (4 tiles)
    vpx_d = nc.dram_tensor("vpx", [NBH, 128, 20, 65], F16, kind="ExternalInput")
    # mask rows for partitions 64-71: [zeros(NSUM) | mq8 | mk8 | dsel | dval]
    # (diag-chunk constants ride in the same tensor: one const DMA at startup)
    msk_d = nc.dram_tensor(
        "msk", [8, 2 * L + NSUM + 128 + GQ], F16, kind="ExternalInput"
    )

    # unnormalized output, transposed: row 64 = softmax denominator l (host divides)
    o_d = nc.dram_tensor("o", [NBH, 65, L], F16, kind="ExternalOutput")

    with tile.TileContext(nc) as tc, ExitStack() as ctx:
        const = ctx.enter_context(tc.tile_pool(name="const", bufs=1))
        inbuf = ctx.enter_context(tc.tile_pool(name="inbuf", bufs=2))
        etsp = ctx.enter_context(tc.tile_pool(name="etsp", bufs=2))
        etlp = ctx.enter_context(tc.tile_pool(name="etlp", bufs=3))
        psum = ctx.enter_context(tc.tile_pool(name="psum", bufs=1, space="PSUM"))
        outp = ctx.enter_context(tc.tile_pool(name="outp", bufs=2))

        # ACT warmup: absorb the exp table load at t~0 (overlaps input DMAs)
        warm = const.tile([128, 8], F32, name="warm")
        nc.vector.memset(warm, 0.0)
        nc.scalar.activation(out=warm, in_=warm, func=EXP)
        # PE warmup: dummy matmuls during the initial DMA wait ramp the PE
        # to full clock (3us continuous-busy threshold) before real work
        warm16 = const.tile([128, 512], F16, name="warm16")
        nc.vector.memset(warm16, 0.0)
        for w in range(7):
            wv = psum.tile([65, GQ], F32, tag="avout", name=f"warm_{w}", bufs=2)
            nc.tensor.matmul(
                wv, warm16[:, 0:65], warm16, start=True, stop=True,
                skip_group_check=True,
            )

        qktb = [
            const.tile([72, 2 * L + NSUM + 128 + GQ], F16, name=f"qktb{j}")
            for j in range(2)
        ]
        # diag constants live in qktb0's mask rows (base partition 64 is a
        # valid stationary tile_position for an 8-row contraction)
        dsel = qktb[0][64:72, 2 * L + NSUM : 2 * L + NSUM + 128]
        dval = qktb[0][64:72, 2 * L + NSUM + 128 :]



        # deferred avg2/avg3 of the previous bh are emitted after the next
        # bh's s0 pieces so PE prioritizes feeding ACT's first exp; the per-bh
        # body is a function so each bh's closures bind their own tiles
        def emit_bh(rep, i, pending):
                qkt = qktb[i % 2]
                first = rep == 0 and i == 0
                if first:
                    nc.sync.dma_start(out=qktb[0][64:72, :], in_=msk_d.ap())
                nc.sync.dma_start(
                    out=qkt[0:64, 0 : 2 * L + NSUM], in_=qkt_d.ap()[i]
                )
                vpx = inbuf.tile([128, 20, 65], F16, tag="vpx")
                nc.sync.dma_start(out=vpx, in_=vpx_d.ap()[i])
                if first:
                    nc.sync.dma_start(out=qktb[1][64:72, :], in_=msk_d.ap())

                qt = qkt[:, NSUM : NSUM + L]      # [72, L] (rows 64-71 = mq8)
                kt = qkt[:, NSUM + L : NSUM + 2 * L]  # (rows 64-71 = mk8)
                qtS = qkt[0:64, NSUM : NSUM + L]
                # gathered summary K^T, per 128-key chunk (contiguous APs --
                # walrus requires single-free-dim stationary operands)
                kv = [qkt[0:64, 128 * s : 128 * (s + 1)] for s in range(4)]

                ets = etsp.tile([128, 5120], F16, tag="ets", name=f"ets_{rep}_{i}")
                # piece layout: s0a 0:1024, s0b 1024:2048, s1a 2048:3072,
                # s1b 3072:3584, s3b 3584:4096, s2a 4096:5120  (s1b+s3b are
                # adjacent so one exp instruction covers both B pieces)
                eoff = [0, 2048, 4096, 3584]

                # summary PSUM split in two ping-pong tiles so each tile's last
                # reader retires early enough for the next bh's QK to overlap
                stA = psum.tile([128, 1024], F32, tag="stA", name=f"sa_{rep}_{i}")
                stB = psum.tile([128, 1024], F32, tag="stB", name=f"sb_{rep}_{i}")
                stloc = psum.tile(
                    [128, 1024], F32, tag="stloc", name=f"sl_{rep}_{i}", bufs=1
                )

                def qk_mms(st, s, q0, nq, diag, toff=0, trim=True):
                    """chunk s scores for queries [q0, q0+nq*GQ) into tile st
                    at column offset toff. The diag piece's first 128 query
                    cols (the group's first window-pair) are fully masked by
                    the pair-causal rule -- skip computing them where the
                    matching exp is also trimmed (trim=False keeps the PSUM
                    fully written for exps that must read the whole range)."""
                    for j in range(nq):
                        tr = 128 if (diag and j == 0 and trim) else 0
                        nc.tensor.matmul(
                            st[:, toff + GQ * j + tr : toff + GQ * (j + 1)],
                            kv[s],
                            qtS[:, q0 + GQ * j + tr : q0 + GQ * (j + 1)],
                            start=True,
                            stop=not (diag and j == 0),
                            skip_group_check=True,
                        )
                    if diag:
                        tr = 128 if trim else 0
                        nc.tensor.matmul(
                            st[:, toff + tr : toff + GQ],
                            dsel,
                            dval[:, tr:GQ],
                            start=False,
                            stop=True,
                            skip_group_check=True,
                        )

                def exp_piece(st, eo, ncols, skip=0):
                    nc.scalar.activation(
                        out=ets[:, eo + skip : eo + ncols],
                        in_=st[:, skip:ncols],
                        func=EXP,
                    )

                def qk_local(j):  # window-pair halves: j=0 -> groups 0,1
                    for h in range(2):
                        g = 2 * j + h
                        for u in range(4):
                            p = 4 * g + u
                            nc.tensor.matmul(
                                stloc[:, GQ * h + 128 * u : GQ * h + 128 * (u + 1)],
                                kt[0:72, 128 * p : 128 * (p + 1)],
                                qt[0:72, 128 * p : 128 * (p + 1)],
                                start=True,
                                stop=True,
                                skip_group_check=True,
                            )

                etl = [None, None]

                def exp_local(j):
                    etl[j] = etlp.tile(
                        [128, 1024], F16, tag="etl", name=f"etl_{rep}_{i}_{j}"
                    )
                    nc.scalar.activation(out=etl[j], in_=stloc, func=EXP)

                osb = outp.tile([65, 4, GQ], F16, tag="osb", name=f"osb_{rep}_{i}")

                def av_local_mm(g, u, av, start):
                    nc.tensor.matmul(
                        av[:, 128 * u : 128 * (u + 1)],
                        vpx[:, 4 * g + u, :],
                        etl[g // 2][:, GQ * (g % 2) + 128 * u :
                                    GQ * (g % 2) + 128 * (u + 1)],
                        start=start,
                        stop=(u == 3),
                        skip_group_check=True,
                    )

                def av_group(g, copy_engine=None):
                    # the diag chunk's (s == g) first 128 query cols were never
                    # computed -- its AV read is trimmed to match. For g > 0
                    # the full-width chunk-0 matmul opens the bank (start=True
                    # zeroes at 2KB-bank granularity); for g == 0 (where chunk
                    # 0 IS trimmed) local u=0 opens it instead.
                    av = psum.tile(
                        [65, GQ], F32, tag="avout", name=f"av_{rep}_{i}_{g}", bufs=2
                    )
                    if g == 0:
                        av_local_mm(0, 0, av, True)
                    for s in range(g + 1):
                        tr = 128 if s == g else 0
                        nc.tensor.matmul(
                            av[:, tr:GQ],
                            vpx[:, 16 + s, :],
                            ets[:, eoff[s] + GQ * (g - s) + tr :
                                   eoff[s] + GQ * (g - s + 1)],
                            start=(s == 0 and g > 0),
                            stop=False,
                            skip_group_check=True,
                        )
                    for u in range(0 if g > 0 else 1, 4):
                        av_local_mm(g, u, av, False)
                    eng = copy_engine or nc.vector
                    eng.tensor_copy(out=osb[:, g, :], in_=av)
                    if rep == reps - 1 and i == NBH - 1:
                        nc.sync.dma_start(
                            out=o_d.ap()[i][:, GQ * g : GQ * (g + 1)],
                            in_=osb[:, g, :],
                        )
                    elif g == 3:
                        dst = o_d.ap()[i].rearrange("p (t c) -> p t c", t=4)
                        nc.sync.dma_start(out=dst, in_=osb)

                # emission order chosen so ACT (the bottleneck) runs back-to-back;
                # ACT order: s0b, s0a, loc01, s1a, {s1b+s3b}, s2a, loc23
                qk_mms(stB, 0, 2 * GQ, 2, False)     # s0b
                exp_piece(stB, 1024, 1024)
                qk_mms(stA, 0, 0, 2, True)           # s0a (diag)
                exp_piece(stA, 0, 1024, skip=128)
                qk_local(0)
                exp_local(0)
                for fn in pending:
                    fn()
                pending = []
                qk_mms(stA, 1, GQ, 2, True)          # s1a (diag)
                exp_piece(stA, 2048, 1024, skip=128)
                qk_mms(stB, 1, 3 * GQ, 1, False)     # s1b at B[0:512]
                qk_mms(stB, 3, 3 * GQ, 1, True, toff=GQ, trim=False)  # s3b
                exp_piece(stB, 3072, 1024)           # {s1b, s3b} one exp
                qk_mms(stA, 2, 2 * GQ, 2, True)      # s2a (diag)
                exp_piece(stA, 4096, 1024, skip=128)
                qk_local(1)
                exp_local(1)
                av_group(0)
                av_group(1)
                return [lambda: av_group(2), lambda: av_group(3)]

        pending = []
        for rep in range(reps):
            for i in range(NBH):
                pending = emit_bh(rep, i, pending)
        for fn in pending:
            fn()

    nc.compile()
    _NC_CACHE[("nc", reps)] = nc
    return nc


def _prep_core_inputs(qf, kf, vf, bhs, mq8, mk8, dsel, dval):
    """Build one core's input dict from flat [32, L, D] fp32 arrays."""
    qkt = np.empty((NBH, 64, 2 * L + NSUM), np.float16)
    vpx = np.empty((NBH, 128, 20, 65), np.float16)
    for j, bh in enumerate(bhs):
        qkt[j, :, 0:NSUM] = kf[bh][_SUMIDX].T.astype(np.float16)
        qkt[j, :, NSUM : NSUM + L] = (qf[bh].T * 0.125).astype(np.float16)
        qkt[j, :, NSUM + L :] = kf[bh].T.astype(np.float16)
        vp1 = np.concatenate([vf[bh], np.ones((L, 1), np.float32)], axis=1).astype(
            np.float16
        )
        vpx[j, :, :16, :] = vp1.reshape(16, 128, 65).transpose(1, 0, 2)
        vs1 = np.concatenate(
            [vf[bh][_SUMIDX], np.ones((NSUM, 1), np.float32)], axis=1
        ).astype(np.float16)
        vpx[j, :, 16:, :] = vs1.reshape(4, 128, 65).transpose(1, 0, 2)
    msk = np.concatenate(
        [np.zeros((8, NSUM), np.float16), mq8, mk8, dsel, dval], axis=1
    )
    return {"qkt": qkt, "vpx": vpx, "msk": msk}


def _finish(o_raw):
    """[n, 65, L] fp16 unnormalized device output -> [n, L, 64] normalized."""
    o_raw = np.asarray(o_raw, np.float32)
    return (o_raw[:, :64, :] / o_raw[:, 64:65, :]).transpose(0, 2, 1)


def _in_maps(query, key, value):
    qf = np.asarray(query, np.float32).reshape(B * H, L, D)
    kf = np.asarray(key, np.float32).reshape(B * H, L, D)
    vf = np.asarray(value, np.float32).reshape(B * H, L, D)
    mq8, mk8, dsel, dval = _host_masks()
    return [
        _prep_core_inputs(
            qf, kf, vf, range(NBH * c, NBH * (c + 1)), mq8, mk8, dsel, dval
        )
        for c in range(NCORES)
    ]


def kernel(query, key, value):
    from concourse.bass_utils import run_bass_kernel_spmd

    nc = _build_nc()
    res = run_bass_kernel_spmd(nc, _in_maps(query, key, value), list(range(NCORES)))
    out = np.concatenate([_finish(res.results[c]["o"]) for c in range(NCORES)])
    return out.reshape(B, H, L, D).astype(np.float32)
